# revision 22
# baseline (speedup 1.0000x reference)
"""Trainium2 Bass kernel for a 4-layer Mamba (BioSeqMixer) model.

Sharding: 8 cores = (batch 4) x (d_inner half 2). Each core runs the full
4-layer stack for one batch element over its 768-channel half of d_inner.
Cross-core traffic: per layer, a pair all-reduce of the x_proj partial
([80,1024]) and of the out_proj partial ([768,1024] fp16).

On-chip layout: channels on partitions, time on the free axis. The selective
scan runs as 16 independent tensor_tensor_scan recurrences (one per state
index n), with dA = exp(A[:,n] * dt) produced by the ACT engine using a
per-partition scale, so A may vary freely over d.
"""

import sys

sys.path.insert(0, "/opt/trn_rl_repo")

import numpy as np

import concourse.bass as bass
import concourse.bacc as bacc
import concourse.mybir as mybir
import concourse.tile as tile
from concourse.bass_utils import run_bass_kernel_spmd

# model dims
B, L = 4, 1024
DM, NL, VOCAB = 768, 4, 8
DI, NST, DCONV, RDT = 1536, 16, 4, 48
EPS = 1e-5

# per-core dims
T = L            # tokens per core (one batch element)
DH = DI // 2     # d_inner half per core
NDC = DH // 128  # d-chunks (6)
NMC = DM // 128  # d_model chunks (6)
NTC = T // 128   # token chunks (8)

N_CORES = 8

F32 = mybir.dt.float32
F16 = mybir.dt.float16
AF = mybir.ActivationFunctionType
ALU = mybir.AluOpType

# knobs
POOL_BCAST = True       # B/C row broadcast on GPSIMD
SCAN_DT = F16           # dtype of dA/uB/h scan tensors
POOL_SCAN_FRAC = 0      # pool cannot run tensor_tensor_scan (ISA reject)
POOL_UB_FRAC = 0        # of 6 d-chunks per n, u*B multiplies on GPSIMD
POOL_YADD_FRAC = 0      # same for the y accumulate add
POOL_CONV = False       # STT not supported on Pool engine
POOL_MISC = False        # u/gate plain tensor_tensor on GPSIMD
DMA_BCAST = False        # broadcast B/C rows via replicated-read DMA
CC_DT = F16             # dtype for the out_proj / x_proj allreduce
SKIP_CC = False         # replace collectives with local copies (for TimelineSim)


def _np16(x):
    return np.ascontiguousarray(x, dtype=np.float16)


def _np32(x):
    return np.ascontiguousarray(x, dtype=np.float32)


def prepare_host_inputs(inputs):
    """Returns per-core input dicts (host-side weight prep + sharding)."""
    embed = np.asarray(inputs["embed"], np.float32)
    input_ids = np.asarray(inputs["input_ids"])
    in_proj_w = np.asarray(inputs["in_proj_w"], np.float32)
    conv_w = np.asarray(inputs["conv_w"], np.float32)
    conv_b = np.asarray(inputs["conv_b"], np.float32)
    x_proj_w = np.asarray(inputs["x_proj_w"], np.float32)
    dt_proj_w = np.asarray(inputs["dt_proj_w"], np.float32)
    dt_proj_b = np.asarray(inputs["dt_proj_b"], np.float32)
    A_log = np.asarray(inputs["A_log"], np.float32)
    Dp = np.asarray(inputs["D"], np.float32)
    out_proj_w = np.asarray(inputs["out_proj_w"], np.float32)
    norm_w = np.asarray(inputs["norm_w"], np.float32)
    norm_b = np.asarray(inputs["norm_b"], np.float32)
    norm_f_w = np.asarray(inputs["norm_f_w"], np.float32)
    norm_f_b = np.asarray(inputs["norm_f_b"], np.float32)

    hidden0 = embed[input_ids]  # (B, L, DM)

    per_half = [{}, {}]
    for h in (0, 1):
        S = slice(h * DH, (h + 1) * DH)
        winx_t = np.empty((NL, DM, DH), np.float16)
        winz_t = np.empty((NL, DM, DH), np.float16)
        bxz = np.empty((NL, 2, DH), np.float32)
        cmat = np.zeros((NL, DCONV, NDC, 128, 128), np.float16)
        dmat = np.zeros((NL, NDC, 128, 128), np.float16)
        convb = np.empty((NL, DH), np.float32)
        wxp_t = np.empty((NL, DH, 80), np.float16)
        wdt_t = np.empty((NL, RDT, DH), np.float16)
        bdt = np.empty((NL, DH), np.float32)
        amat = np.empty((NL, DH, NST), np.float32)
        wout_t = np.empty((NL, DH, DM), np.float16)
        for l in range(NL):
            wx_rows = in_proj_w[l][:DI][S]          # (DH, DM)
            wz_rows = in_proj_w[l][DI:][S]          # (DH, DM)
            winx_t[l] = _np16((wx_rows * norm_w[l][None, :]).T)
            winz_t[l] = _np16((wz_rows * norm_w[l][None, :]).T)
            bxz[l, 0] = wx_rows @ norm_b[l]
            bxz[l, 1] = wz_rows @ norm_b[l]
            cw = conv_w[l][S]                       # (DH, DCONV)
            dv = Dp[l][S]
            ii = np.arange(128)
            for dc in range(NDC):
                rows = slice(dc * 128, (dc + 1) * 128)
                for k in range(DCONV):
                    cmat[l, k, dc, ii, ii] = cw[rows, k]
                dmat[l, dc, ii, ii] = dv[rows]
            convb[l] = conv_b[l][S]
            wxp_t[l] = _np16(x_proj_w[l][:, S].T)   # (DH, 80)
            wdt_t[l] = _np16(dt_proj_w[l][S].T)     # (RDT, DH)
            bdt[l] = dt_proj_b[l][S]
            amat[l] = -np.exp(A_log[l][S])          # (DH, NST)
            wout_t[l] = _np16(out_proj_w[l][:, S].T)  # (DH, DM)
        per_half[h] = dict(
            winx_t=winx_t, winz_t=winz_t, bxz=bxz, convb=convb,
            cmat=cmat.reshape(NL, DCONV * NDC * 128, 128),
            dmat=dmat.reshape(NL, NDC * 128, 128),
            wxp_t=wxp_t, wdt_t=wdt_t, bdt=bdt, amat=amat,
            wout_t=wout_t,
        )

    wfin = _np16(np.tile(norm_f_w[None, :], (128, 1)))
    bfin = _np16(np.tile(norm_f_b[None, :], (128, 1)))
    ident = _np16(np.eye(128))

    in_maps = []
    for r in range(N_CORES):
        b, h = r // 2, r % 2
        m = dict(per_half[h])
        m = {k: np.ascontiguousarray(v) for k, v in m.items()}
        m["hidden0"] = _np32(hidden0[b])
        m["wfin"] = wfin
        m["bfin"] = bfin
        m["ident"] = ident
        in_maps.append(m)
    return in_maps


def build_program():
    nc = bacc.Bacc("TRN2", target_bir_lowering=False, debug=False,
                   num_devices=N_CORES)

    dt_in = {}

    def din(name, shape, dt=F32):
        dt_in[name] = nc.dram_tensor(name, list(shape), dt,
                                     kind="ExternalInput").ap()
        return dt_in[name]

    din("hidden0", (T, DM))
    din("winx_t", (NL, DM, DH), F16)
    din("winz_t", (NL, DM, DH), F16)
    din("bxz", (NL, 2, DH))
    din("cmat", (NL, DCONV * NDC * 128, 128), F16)
    din("dmat", (NL, NDC * 128, 128), F16)
    din("convb", (NL, DH))
    din("wxp_t", (NL, DH, 80), F16)
    din("wdt_t", (NL, RDT, DH), F16)
    din("bdt", (NL, DH))
    din("amat", (NL, DH, NST))
    din("wout_t", (NL, DH, DM), F16)
    din("wfin", (128, DM), F16)
    din("bfin", (128, DM), F16)
    din("ident", (128, 128), F16)

    out_ap = nc.dram_tensor("out", [T, DM], F32, kind="ExternalOutput").ap()

    with tile.TileContext(nc) as tc:
        _body(nc, tc, dt_in, out_ap)

    nc.compile()
    return nc


def _body(nc, tc, din, out_ap):
    import contextlib
    with contextlib.ExitStack() as ctx:
        _body_inner(ctx, nc, tc, din, out_ap)


def _rep_mid(ap, n):
    """[P, T] AP -> [P, n, T] with a step-0 broadcast middle dim."""
    return bass.AP(ap.tensor, ap.offset, [ap.ap[0], [0, n], ap.ap[1]])


def _ln_stats(nc, res, stats, sq, epsc):
    """Compute per token-chunk mean (mu) and 1/std (rstd) into stats cols."""
    nc.vector.tensor_reduce(stats[:, 0:NTC],
                            res[:].rearrange("p (c m) -> p c m", c=NTC),
                            mybir.AxisListType.X, ALU.add)
    for c in range(NTC):
        rc = res[:, c * DM:(c + 1) * DM]
        nc.scalar.activation(sq[:, :DM], rc, AF.Square,
                             accum_out=stats[:, NTC + c:NTC + c + 1])
    mu = stats[:, 2 * NTC:3 * NTC]
    rstd = stats[:, 3 * NTC:4 * NTC]
    nc.vector.tensor_scalar(mu, stats[:, 0:NTC], 1.0 / DM, None, ALU.mult)
    nc.vector.tensor_tensor(rstd, mu, mu, ALU.mult)
    nc.vector.scalar_tensor_tensor(rstd, stats[:, NTC:2 * NTC], 1.0 / DM,
                                   rstd, ALU.mult, ALU.subtract)
    nc.scalar.activation(rstd, rstd, AF.Sqrt, bias=epsc[:])
    nc.vector.reciprocal(rstd, rstd)
    return mu, rstd


def _body_inner(ctx, nc, tc, din, out_ap):
    E = ctx.enter_context

    # pools
    persist = E(tc.tile_pool(name="persist", bufs=1))
    wpool = E(tc.tile_pool(name="weights", bufs=2))
    wsmall = E(tc.tile_pool(name="wsmall", bufs=1))
    lnt_pool = E(tc.tile_pool(name="lnt", bufs=1))
    scratch = E(tc.tile_pool(name="scratch", bufs=2))
    t32 = E(tc.tile_pool(name="t32", bufs=2))      # dA pair transients
    ubh = E(tc.tile_pool(name="ubh", bufs=4))      # uB/h half-rows
    bcp = E(tc.tile_pool(name="bcp", bufs=3))      # B/C broadcast rows
    bcst = E(tc.tile_pool(name="bcst", bufs=2))    # 1-partition staging rows
    smalls = E(tc.tile_pool(name="smalls", bufs=1))
    dram = E(tc.tile_pool(name="dram", bufs=2, space="DRAM"))

    # persistent tiles
    res = persist.tile([128, NTC * DM], F32, tag="res")        # residual [t,dm]
    xbuf = persist.tile([128, NDC * (T + 3)], F16, tag="xbuf")  # conv in, 3-pad
    xs = persist.tile([128, NDC * T], F16, tag="xs")           # silu(conv(x))
    zs = persist.tile([128, NDC * T], F16, tag="zs")           # silu(z)
    dts = persist.tile([128, NDC * T], F16, tag="dts")         # softplus dt
    u = persist.tile([128, NDC * T], F16, tag="u")             # dt * xs
    y = persist.tile([128, NDC * T], F16, tag="y")             # scan output acc
    dbcall = persist.tile([80, T], F16, tag="dbcall")
    ident_sb = persist.tile([128, 128], F16, tag="ident")
    wfin_sb = persist.tile([128, DM], F16, tag="wfin")
    bfin_sb = persist.tile([128, DM], F16, tag="bfin")
    epsc = persist.tile([128, 1], F32, tag="epsc")

    nc.vector.memset(epsc[:], EPS)
    nc.sync.dma_start(ident_sb[:], din["ident"][:, :])
    nc.sync.dma_start(wfin_sb[:], din["wfin"][:, :])
    nc.sync.dma_start(bfin_sb[:], din["bfin"][:, :])

    # residual <- hidden0 ([T, DM] -> [128, (tc dm)])
    nc.sync.dma_start(
        res[:].rearrange("p (c m) -> p c m", c=NTC),
        din["hidden0"].rearrange("(c p) m -> p c m", p=128))

    # zero the 3-column conv pads once
    for dc in range(NDC):
        nc.vector.memset(xbuf[:, dc * (T + 3): dc * (T + 3) + 3], 0.0)

    kw = dict(res=res, xbuf=xbuf, xs=xs, zs=zs, dts=dts, u=u, y=y,
              dbcall=dbcall, ident_sb=ident_sb, epsc=epsc,
              wpool=wpool, wsmall=wsmall, lnt_pool=lnt_pool, scratch=scratch,
              t32=t32, ubh=ubh, bcp=bcp, bcst=bcst, smalls=smalls, dram=dram)
    for layer in range(NL):
        _layer(nc, tc, din, layer, **kw)

    # final layernorm -> out
    stats = smalls.tile([128, 4 * NTC], F32, tag="stats")
    sq = scratch.tile([128, T], F16, tag="sq")
    mu, rstd = _ln_stats(nc, res, stats, sq, epsc)
    for c in range(NTC):
        rc = res[:, c * DM:(c + 1) * DM]
        ot = scratch.tile([128, DM], F32, tag="lnout", bufs=1)
        nc.vector.tensor_scalar(ot[:], rc, mu[:, c:c + 1], rstd[:, c:c + 1],
                                ALU.subtract, ALU.mult)
        nc.vector.tensor_tensor(ot[:], ot[:], wfin_sb[:], ALU.mult)
        nc.vector.tensor_tensor(ot[:], ot[:], bfin_sb[:], ALU.add)
        nc.sync.dma_start(out_ap[c * 128:(c + 1) * 128, :], ot[:])


def _layer(nc, tc, din, layer, *, res, xbuf, xs, zs, dts, u, y, dbcall,
           ident_sb, epsc, wpool, wsmall, lnt_pool, scratch, t32, ubh, bcp,
           bcst, smalls, dram):
    lt = lambda name: din[name][layer]

    def load3(tile_ap, dram_ap, k):
        nc.sync.dma_start(
            tile_ap.rearrange("p (k m) -> p k m", k=k),
            dram_ap.rearrange("(k p) m -> p k m", p=128))

    # --- load weights to sbuf ---
    winx = wpool.tile([128, NMC * DH], F16, tag="wbig")
    load3(winx[:], lt("winx_t"), NMC)
    winz = wpool.tile([128, NMC * DH], F16, tag="wbig")
    load3(winz[:], lt("winz_t"), NMC)
    wxp = wsmall.tile([128, NDC * 80], F16, tag="wxp")
    load3(wxp[:], lt("wxp_t"), NDC)
    wdt = wsmall.tile([RDT, DH], F16, tag="wdt")
    nc.sync.dma_start(wdt[:], lt("wdt_t")[:, :])
    amat = wsmall.tile([128, NDC * NST], F32, tag="amat")
    load3(amat[:], lt("amat"), NDC)
    cmat = wsmall.tile([128, DCONV * NDC * 128], F16, tag="cmat")
    load3(cmat[:], lt("cmat"), DCONV * NDC)
    dmat = wsmall.tile([128, NDC * 128], F16, tag="dmat")
    load3(dmat[:], lt("dmat"), NDC)
    vecs = wsmall.tile([128, NDC * 4], F32, tag="vecs")
    # layout per dchunk: [bx, bz, convb, bdt]
    nc.sync.dma_start(
        vecs[:, 0:NDC * 2].rearrange("p (b k) -> p b k", b=2),
        lt("bxz").rearrange("b (k p) -> p b k", p=128))
    nc.sync.dma_start(vecs[:, NDC * 2:NDC * 3],
                      lt("convb").rearrange("(k p) -> p k", p=128))
    nc.sync.dma_start(vecs[:, NDC * 3:NDC * 4],
                      lt("bdt").rearrange("(k p) -> p k", p=128))
    bx_c = lambda dc: vecs[:, dc:dc + 1]
    bz_c = lambda dc: vecs[:, NDC + dc:NDC + dc + 1]
    convb_c = lambda dc: vecs[:, NDC * 2 + dc:NDC * 2 + dc + 1]
    bdt_c = lambda dc: vecs[:, NDC * 3 + dc:NDC * 3 + dc + 1]

    with tc.tile_pool(name="pp_a", bufs=4, space="PSUM") as pp_a:
        # --- layernorm stats over residual ---
        stats = smalls.tile([128, 4 * NTC], F32, tag="stats")
        sq = scratch.tile([128, T], F16, tag="sq")
        mu, rstd = _ln_stats(nc, res, stats, sq, epsc)

        # --- ln apply + transpose -> lnT [dm, t] f16 ---
        lnT = lnt_pool.tile([128, NMC * T], F16, tag="lnT")
        lnT3 = lnT[:].rearrange("p (m t) -> p m t", m=NMC)
        for c in range(NTC):
            rc = res[:, c * DM:(c + 1) * DM]
            lnt = scratch.tile([128, DM], F16, tag="lnapply")
            nc.vector.tensor_scalar(lnt[:], rc, mu[:, c:c + 1],
                                    rstd[:, c:c + 1],
                                    ALU.subtract, ALU.mult)
            ptr = pp_a.tile([128, DM], F16, tag="tr", bufs=2)
            for mc in range(NMC):
                nc.tensor.transpose(ptr[:, mc * 128:(mc + 1) * 128],
                                    lnt[:, mc * 128:(mc + 1) * 128],
                                    ident_sb[:])
            nc.scalar.copy(lnT3[:, :, c * 128:(c + 1) * 128],
                           ptr[:].rearrange("p (m t) -> p m t", m=NMC))

        # --- in_proj matmuls: x rows then z rows ---
        for xz in range(2):
            wmat = winx if xz == 0 else winz
            for dc in range(NDC):
                for nh in range(2):
                    pm = pp_a.tile([128, 512], F32, tag="mm")
                    for k in range(NMC):
                        nc.tensor.matmul(
                            pm[:],
                            wmat[:, k * DH + dc * 128: k * DH + (dc + 1) * 128],
                            lnT[:, k * T + nh * 512: k * T + (nh + 1) * 512],
                            start=(k == 0), stop=(k == NMC - 1))
                    if xz == 0:
                        dst = xbuf[:, dc * (T + 3) + 3 + nh * 512:
                                   dc * (T + 3) + 3 + (nh + 1) * 512]
                        nc.scalar.activation(dst, pm[:], AF.Identity,
                                             bias=bx_c(dc))
                    else:
                        dst = zs[:, dc * T + nh * 512: dc * T + (nh + 1) * 512]
                        nc.scalar.activation(dst, pm[:], AF.Silu,
                                             bias=bz_c(dc))

        # --- conv on PE: 4 diag matmuls per tile, silu evacuates ---
        for dc in range(NDC):
            x0 = dc * (T + 3)
            for nh in range(2):
                pm = pp_a.tile([128, 512], F32, tag="mm")
                for k in range(DCONV):
                    nc.tensor.matmul(
                        pm[:], cmat[:, (k * NDC + dc) * 128:
                                    (k * NDC + dc + 1) * 128],
                        xbuf[:, x0 + k + nh * 512: x0 + k + nh * 512 + 512],
                        start=(k == 0), stop=(k == DCONV - 1))
                nc.scalar.activation(xs[:, dc * T + nh * 512:
                                        dc * T + (nh + 1) * 512],
                                     pm[:], AF.Silu, bias=convb_c(dc))

        # --- x_proj -> dbc partial -> pair allreduce -> dbcall (one DMA) ---
        dbc_p = scratch.tile([80, T], CC_DT, tag="dbcp")
        for nh in range(2):
            pm = pp_a.tile([80, 512], F32, tag="mm")
            for k in range(NDC):
                nc.tensor.matmul(
                    pm[:], wxp[:, k * 80:(k + 1) * 80],
                    xs[:, k * T + nh * 512: k * T + (nh + 1) * 512],
                    start=(k == 0), stop=(k == NDC - 1))
            nc.scalar.copy(dbc_p[:, nh * 512:(nh + 1) * 512], pm[:])
        db_in = dram.tile([80, T], CC_DT, tag="db_in")
        db_out = dram.tile([80, T], CC_DT, tag="db_out")
        nc.sync.dma_start(db_in[:], dbc_p[:])
        if SKIP_CC:
            nc.sync.dma_start(db_out[:], db_in[:])
        else:
            nc.gpsimd.collective_compute(
                "AllReduce", ALU.add,
                replica_groups=[[0, 1], [2, 3], [4, 5], [6, 7]],
                ins=[db_in.opt()], outs=[db_out.opt()])
        nc.sync.dma_start(dbcall[:], db_out[:, :])

        # --- dt_proj -> softplus -> dts ---
        # softplus = ln(1 + e^w): all 12 Exp ops run contiguously (into u
        # as scratch), then one full-width Ln — at most 2 act-table loads
        # instead of one per Exp<->Ln alternation.
        for dc in range(NDC):
            for nh in range(2):
                pm = pp_a.tile([128, 512], F32, tag="mm")
                nc.tensor.matmul(pm[:], wdt[:, dc * 128:(dc + 1) * 128],
                                 dbcall[0:RDT, nh * 512:(nh + 1) * 512],
                                 start=True, stop=True)
                dst = u[:, dc * T + nh * 512: dc * T + (nh + 1) * 512]
                nc.scalar.activation(dst, pm[:], AF.Exp, bias=bdt_c(dc))
        nc.scalar.activation(dts[:], u[:], AF.Ln, bias=1.0)

    # --- u = dts * xs (one full-width op) ---
    nc.vector.tensor_tensor(u[:], dts[:], xs[:], ALU.mult)

    # --- scan over (d-half, n): uB / h*C on DVE, y = sum_n h*C + D*xs
    # accumulated on PE (identity / diag-D matmuls into a PSUM tile).
    NH2 = NDC // 2  # chunks per half (3)
    with tc.tile_pool(name="pp_y", bufs=1, space="PSUM") as pp_y:
        for half in range(2):
            base = half * NH2
            u3h = u[:, base * T:(base + NH2) * T].rearrange(
                "p (d t) -> p d t", d=NH2)
            ypsum = pp_y.tile([128, NH2 * T], F32, tag="ypsum")
            for n in range(NST):
                bn = bcp.tile([128, T], F16, tag="bc")
                cn = bcp.tile([128, T], F16, tag="bc")
                brow = bcst.tile([1, T], F16, tag="bcst")
                nc.sync.dma_start(brow[:], db_out[RDT + n:RDT + n + 1, :])
                nc.gpsimd.partition_broadcast(bn[:], brow[:])
                crow = bcst.tile([1, T], F16, tag="bcst")
                nc.sync.dma_start(
                    crow[:], db_out[RDT + NST + n:RDT + NST + n + 1, :])
                nc.gpsimd.partition_broadcast(cn[:], crow[:])
                uball = ubh.tile([128, NH2 * T], SCAN_DT, tag="ubh")
                ub3 = uball[:].rearrange("p (d t) -> p d t", d=NH2)
                nc.vector.tensor_tensor(ub3, u3h, _rep_mid(bn[:], NH2),
                                        ALU.mult)
                hall = ubh.tile([128, NH2 * T], SCAN_DT, tag="ubh")
                # chunks (0,1) share one scan with a dA=0 segment reset
                # (exact: each chunk's recurrence starts from zero state);
                # chunk 2 scans alone.
                da = t32.tile([128, 2 * T], SCAN_DT, tag="t32")
                for j in range(2):
                    dc = base + j
                    nc.scalar.activation(
                        da[:, j * T:(j + 1) * T], dts[:, dc * T:(dc + 1) * T],
                        AF.Exp, scale=amat[:, dc * NST + n:dc * NST + n + 1])
                nc.gpsimd.memset(da[:, T:T + 1], 0.0)
                nc.vector.tensor_tensor_scan(
                    hall[:, 0:2 * T], da[:], uball[:, 0:2 * T],
                    0.0, ALU.mult, ALU.add)
                das = t32.tile([128, T], SCAN_DT, tag="t32s")
                dc = base + 2
                nc.scalar.activation(
                    das[:], dts[:, dc * T:(dc + 1) * T],
                    AF.Exp, scale=amat[:, dc * NST + n:dc * NST + n + 1])
                nc.vector.tensor_tensor_scan(
                    hall[:, 2 * T:3 * T], das[:], uball[:, 2 * T:3 * T],
                    0.0, ALU.mult, ALU.add)
                # hc = h * C -> back into uball, then PE accumulates
                h3 = hall[:].rearrange("p (d t) -> p d t", d=NH2)
                nc.vector.tensor_tensor(ub3, h3, _rep_mid(cn[:], NH2),
                                        ALU.mult)
                for j6 in range(2 * NH2):
                    nc.tensor.matmul(
                        ypsum[:, j6 * 512:(j6 + 1) * 512], ident_sb[:],
                        uball[:, j6 * 512:(j6 + 1) * 512],
                        start=(n == 0), stop=False)
            # D*xs closes each accumulation bank
            for j6 in range(2 * NH2):
                dc = base + j6 // 2
                nc.tensor.matmul(
                    ypsum[:, j6 * 512:(j6 + 1) * 512],
                    dmat[:, dc * 128:(dc + 1) * 128],
                    xs[:, base * T + j6 * 512: base * T + (j6 + 1) * 512],
                    start=False, stop=True)
            # evacuate on ACT into y, gate with silu(z) in place on DVE
            ysl = y[:, base * T:(base + NH2) * T]
            nc.scalar.copy(ysl, ypsum[:])
            nc.vector.tensor_tensor(ysl, ysl,
                                    zs[:, base * T:(base + NH2) * T],
                                    ALU.mult)

    # --- out_proj -> partial -> pair allreduce ---
    wout = wpool.tile([128, NDC * DM], F16, tag="wbig")
    load3(wout[:], lt("wout_t"), NDC)
    op_in = dram.tile([128, NMC * T], CC_DT, tag="op_in")
    op_out = dram.tile([128, NMC * T], CC_DT, tag="op_out")
    with tc.tile_pool(name="pp_o", bufs=4, space="PSUM") as pp_o:
        for mc in range(NMC):
            stg = scratch.tile([128, T], CC_DT, tag="opstg")
            for nh in range(2):
                pm = pp_o.tile([128, 512], F32, tag="mm")
                for k in range(NDC):
                    nc.tensor.matmul(
                        pm[:],
                        wout[:, k * DM + mc * 128: k * DM + (mc + 1) * 128],
                        y[:, k * T + nh * 512: k * T + (nh + 1) * 512],
                        start=(k == 0), stop=(k == NDC - 1))
                nc.scalar.copy(stg[:, nh * 512:(nh + 1) * 512], pm[:])
            nc.sync.dma_start(op_in[:, mc * T:(mc + 1) * T], stg[:])
        if SKIP_CC:
            nc.sync.dma_start(op_out[:], op_in[:])
        else:
            nc.gpsimd.collective_compute(
                "AllReduce", ALU.add,
                replica_groups=[[0, 1], [2, 3], [4, 5], [6, 7]],
                ins=[op_in.opt()], outs=[op_out.opt()])

        # --- transpose opf back to [t, dm] and add to residual ---
        # 8 token-chunk transposes land in one [128,1024] psum tile; a
        # single 3D-AP tensor_tensor adds them into the strided residual
        # columns.
        res3 = res[:].rearrange("p (c m) -> p c m", c=NTC)
        for mc in range(NMC):
            opc = scratch.tile([128, T], CC_DT, tag="opstg")
            nc.sync.dma_start(opc[:], op_out[:, mc * T:(mc + 1) * T])
            ptr = pp_o.tile([128, T], CC_DT, tag="tr", bufs=2)
            for c in range(NTC):
                nc.tensor.transpose(ptr[:, c * 128:(c + 1) * 128],
                                    opc[:, c * 128:(c + 1) * 128],
                                    ident_sb[:])
            rsl = res3[:, :, mc * 128:(mc + 1) * 128]
            ptr3 = ptr[:].rearrange("p (c m) -> p c m", c=NTC)
            nc.vector.tensor_tensor(rsl, rsl, ptr3, ALU.add)


_PROGRAM = None


def kernel(**inputs):
    return kernel_ex(inputs)[0]


def kernel_ex(inputs, trace=False):
    import os
    global _PROGRAM
    in_maps = prepare_host_inputs(inputs)
    if _PROGRAM is None:
        _PROGRAM = build_program()
    kwargs = {}
    if trace:
        kwargs = dict(trace=True)
    res = run_bass_kernel_spmd(_PROGRAM, in_maps,
                               core_ids=list(range(N_CORES)), **kwargs)
    out = np.empty((B, L, DM), np.float32)
    for b in range(B):
        out[b] = res.results[2 * b]["out"]
    return out, res

